# revision 1
# baseline (speedup 1.0000x reference)
"""DechirpSTFT Trainium2 kernel.

Math: the reference pipeline (hann window -> per-chirp lerp resample * jac
-> rfft(1024)) is linear in the windowed signal, so it folds into one
per-chirp matrix G_d[k, f2] (k = sample within window, f2 = interleaved
re/im of the 513 rfft bins).  The device kernel is the dense matmul

    out[f2, w] = sum_k G_d[k, f2] * x[b, 512*w + k]

computed G-stationary: each 128x128 tile of G is the PE weight operand and
all 512 (padded) windows stream through as the moving operand, contracting
k over 8 chunks of 128 into one PSUM bank per (b, f2-tile).

Key HW findings baked in (measured on the axon trn2 cores):
  * The PE streams ~2 cols/ns (warm 2.4 GHz) ONLY when the moving operand
    is contiguous in SBUF; a stride-4 free-axis slice halves throughput
    and a strided weight load makes LDWEIGHTS the bottleneck.  Therefore
    x is shipped as xt[p, b, r, mq] = x[b, 128*(4*mq+r)+p] so every
    matmul slice xt[:, b, r, q:q+512] is stride-1.
  * fp32r at N>=256 streams at bf16 rate with fp32-grade accuracy
    (rel err 1.5e-4); bf16/bf16-out gave no speedup (PE-bound, DMA
    overlapped) so operands and output stay fp32.
  * fp32r requires an even moving-operand count: windows padded 511->512
    (xt padded to 513 m-quads; host drops the garbage window).
  * im(bin 0) and im(Nyquist) columns of G are identically zero, so each
    chirp has exactly 1024 live f2 columns -> per core (2 chirps) exactly
    16 weight tiles of 128.  Host re-inserts the zero columns.

Sharding: D=16 chirp rates, 2 per core across 8 cores (x replicated,
G sliced per core).  No cross-core communication.

Output DRAM layout is [B, ftg, p, j, w] (f2-tile-major, windows innermost,
matching PSUM orientation); the host transposes back to (B, NW, D, 513)
complex64.  Host-side prep/assembly time is not part of device exec time.

Schedule: loads ordered so the first chain's deps land first (x[b0],
all g, then x[b1..3]); x and g pools double-buffered across For_i
iterations (~26.0 of 26.6 MB SBUF — every load overlaps prior compute);
each 128x512 PSUM tile filled by two h-outer N=256 chains (single
start/stop — `start` clears has_written bank-wide) and DMA'd per f2-tile
right after its eviction.  An interleaved same-session A/B measured this
~20 us/iter faster than the plain b-outer N=512 schedule.

Timing: no NTFF profiling is reachable through this axon client, so
test.py measures HW time as (wall(For_i(T)) - wall(For_i(1))) / (T-1)
with the jitted executable and device-resident inputs held across calls.
HW exec time measured at this commit: ~112-126 us/iter (vs 616 us
stated / ~348 us re-measured baseline).
"""

import os
import sys

sys.path.insert(0, "/opt/trn_rl_repo")

import numpy as np

# ---- problem constants (hardcoded; kernel.py must be self-contained) ----
B = 4
N = 262144
K = 1024
HOP = 512
NW = (N - K) // HOP + 1          # 511
KTAU = 1024
NF = 513                         # rfft bins
FW = 2 * NF                      # 1026 interleaved re/im
FE = 1024                        # live f2 cols per chirp (im0/imNyq dropped)
NWP = 512                        # padded window count (fp32r needs even N)
MQP = 513                        # padded m-quads so window 511 stays in-bounds
D = 16
NCORES = 8
D_PER = D // NCORES              # 2
KC = 8                           # contraction chunks of 128
MQ = N // 512                    # 512
NFT = 16                         # f2e weight tiles of 128 per core
EPS = 1e-8

_cache = {}


# --------------------------------------------------------------------------
# host-side G construction
# --------------------------------------------------------------------------
def _build_tables_np(dlnf):
    """Reference's per-chirp tables in numpy float32 (fallback path)."""
    dlnf = dlnf.astype(np.float32)
    beta = (2.0 * dlnf).astype(np.float32)
    small = np.abs(beta) < EPS
    beta_safe = np.where(small, np.float32(EPS), beta).astype(np.float32)
    e2b = np.exp(2.0 * beta_safe).astype(np.float32)

    tau = (2.0 * np.arange(KTAU, dtype=np.float32) / KTAU - 1.0).astype(np.float32)
    t_source = np.log(
        1.0 + (tau[None, :] + 1.0) / 2.0 * (e2b[:, None] - 1.0)
    ).astype(np.float32)
    t_source = (t_source / beta_safe[:, None] - 1.0).astype(np.float32)
    t_source = np.where(small[:, None], tau[None, :], t_source)

    tau_mid = np.float32(2.0 * (KTAU // 2) / KTAU - 1.0)
    t_mid = (
        np.log(1.0 + (tau_mid + 1.0) / 2.0 * (e2b - 1.0)) / beta_safe - 1.0
    ).astype(np.float32)
    t_mid = np.where(small, tau_mid, t_mid)

    jac = np.exp(-beta_safe[:, None] * (t_source - t_mid[:, None])).astype(np.float32)
    jac = np.where(small[:, None], np.float32(1.0), jac)

    idx = (np.float32(K / 2.0) * (t_source + 1.0)).astype(np.float32)
    idx_lo = np.clip(idx.astype(np.int32), 0, K - 2)
    frac = (idx - idx_lo.astype(np.float32)).astype(np.float32)
    return idx_lo, frac, jac


def _build_tables(dlnf):
    """Per-chirp tables, computed with jax on the CPU backend so the f32
    transcendentals (log/exp) match the reference bit-for-bit."""
    try:
        import jax
        import jax.numpy as jnp

        cpu = jax.devices("cpu")[0]
    except Exception:
        return _build_tables_np(dlnf)

    with jax.default_device(cpu):
        beta = 2.0 * jnp.asarray(dlnf, dtype=jnp.float32)
        small = jnp.abs(beta) < EPS
        beta_safe = jnp.where(small, EPS, beta)
        e2b = jnp.exp(2.0 * beta_safe)

        tau = 2.0 * jnp.arange(KTAU, dtype=jnp.float32) / KTAU - 1.0
        t_source = (
            jnp.log(1.0 + (tau[None, :] + 1.0) / 2.0 * (e2b[:, None] - 1.0))
            / beta_safe[:, None]
            - 1.0
        )
        t_source = jnp.where(small[:, None], tau[None, :], t_source)

        tau_mid = 2.0 * (KTAU // 2) / KTAU - 1.0
        t_mid = (
            jnp.log(1.0 + (tau_mid + 1.0) / 2.0 * (e2b - 1.0)) / beta_safe - 1.0
        )
        t_mid = jnp.where(small, tau_mid, t_mid)

        jac = jnp.exp(-beta_safe[:, None] * (t_source - t_mid[:, None]))
        jac = jnp.where(small[:, None], 1.0, jac)

        idx = (K / 2.0) * (t_source + 1.0)
        idx_lo = jnp.clip(idx.astype(jnp.int32), 0, K - 2)
        frac = idx - idx_lo.astype(jnp.float32)
    return np.asarray(idx_lo), np.asarray(frac), np.asarray(jac)


def _build_G(dlnf):
    """G[d, k, f2] f32: fused hann * lerp-resample * jac * rfft operator."""
    nd = dlnf.shape[0]
    idx_lo, frac, jac = _build_tables(dlnf)
    t = np.arange(KTAU, dtype=np.float64)
    f = np.arange(NF, dtype=np.float64)
    ang = 2.0 * np.pi * np.outer(t, f) / KTAU
    Wre = np.cos(ang)
    Wim = -np.sin(ang)
    n = np.arange(K, dtype=np.float32)
    hann = (0.5 * (1.0 - np.cos(2.0 * np.pi * n / K))).astype(np.float32)

    G = np.zeros((nd, K, FW), dtype=np.float64)
    for d in range(nd):
        c_lo = (jac[d] * (1.0 - frac[d])).astype(np.float64)
        c_hi = (jac[d] * frac[d]).astype(np.float64)
        Gre = np.zeros((K, NF))
        Gim = np.zeros((K, NF))
        np.add.at(Gre, idx_lo[d], c_lo[:, None] * Wre)
        np.add.at(Gim, idx_lo[d], c_lo[:, None] * Wim)
        np.add.at(Gre, idx_lo[d] + 1, c_hi[:, None] * Wre)
        np.add.at(Gim, idx_lo[d] + 1, c_hi[:, None] * Wim)
        G[d, :, 0::2] = Gre
        G[d, :, 1::2] = Gim
    G *= hann[None, :, None].astype(np.float64)
    return G.astype(np.float32)


# live G columns: drop im(bin0) (col 1) and im(Nyquist) (col 1025)
_KEEP = np.concatenate(([0], np.arange(2, 1025)))


# --------------------------------------------------------------------------
# device program
# --------------------------------------------------------------------------
def _build_nc(iters=1):
    import concourse.bacc as bacc
    import concourse.mybir as mybir
    from concourse import tile

    mm_dt = mybir.dt.float32r
    f32 = mybir.dt.float32

    nc = bacc.Bacc("TRN2", target_bir_lowering=False, debug=False)

    # xt[p, b, r, mq] = x[b, 128*(4*mq + r) + p]  (mq innermost: every
    # matmul moving slice is contiguous)
    xt_d = nc.dram_tensor("xt", [128, B, 4, MQP], mm_dt, kind="ExternalInput")
    # g[p, kc, 1024*d + fe] = G_d[128*kc + p, keep[fe]]
    g_d = nc.dram_tensor("g", [128, KC, D_PER * FE], mm_dt, kind="ExternalInput")
    # out2[b, ftg, p, j, w]: f2e tile ft = 4*ftg + j, psum partition p
    out_d = nc.dram_tensor(
        "out", [B, NFT // 4, 128, 4, NWP], f32, kind="ExternalOutput"
    )

    def body(nc, tc, xpool, gpool, spool, ppool):
        x_sb = xpool.tile([128, B, 4, MQP], mm_dt, name="x_sb")
        g_sb = gpool.tile([128, KC, D_PER * FE], mm_dt, name="g_sb")
        for b in range(B):
            nc.sync.dma_start(x_sb[:, b], xt_d[:, b])
        for kc in range(KC):
            nc.sync.dma_start(g_sb[:, kc], g_d[:, kc])

        for b in range(B):
            for ftp in range(NFT // 2):
                st = spool.tile([128, 2, NWP], f32, name="st")
                for jj in range(2):
                    ft = 2 * ftp + jj
                    ps = ppool.tile([128, NWP], f32, name="ps", tag="ps")
                    # two h-outer N=256 chains into one bank: one start/stop
                    for h in range(2):
                        for kc in range(KC):
                            q, r = divmod(kc, 4)
                            nc.tensor.matmul(
                                ps[:, h * 256 : (h + 1) * 256],
                                g_sb[:, kc, 128 * ft : 128 * (ft + 1)],
                                x_sb[:, b, r, q + h * 256 : q + h * 256 + 256],
                                start=(kc == 0 and h == 0),
                                stop=(kc == KC - 1 and h == 1),
                            )
                    eng = nc.vector.tensor_copy if ft % 2 == 0 else nc.scalar.copy
                    eng(st[:, jj], ps)
                # stores issue from the ACT HWDGE ring: HWDGE executes FIFO
                # per issuing engine, so next-iteration loads (sync ring)
                # don't queue behind this iteration's stores
                nc.scalar.dma_start(
                    out_d[b, ftp // 2, :, 2 * (ftp % 2) : 2 * (ftp % 2) + 2], st
                )

    with tile.TileContext(nc) as tc:
        with (
            tc.tile_pool(name="xsb", bufs=2) as xpool,
            tc.tile_pool(name="gsb", bufs=2) as gpool,
            tc.tile_pool(name="stage", bufs=3) as spool,
            tc.tile_pool(name="psum", bufs=8, space="PSUM") as ppool,
        ):
            if iters > 1:
                with tc.For_i(0, iters, 1):
                    body(nc, tc, xpool, gpool, spool, ppool)
            else:
                body(nc, tc, xpool, gpool, spool, ppool)

    nc.compile()
    return nc


def _get_nc(iters=1):
    key = ("nc", iters)
    if key not in _cache:
        _cache[key] = _build_nc(iters)
    return _cache[key]


# --------------------------------------------------------------------------
# host prep / assembly
# --------------------------------------------------------------------------
def _prep_arrays(x, dlnf):
    """Host prep: G matrices + transposed/sharded device input arrays."""
    x = np.asarray(x, dtype=np.float32)
    dlnf = np.asarray(dlnf, dtype=np.float32)
    G = _build_G(dlnf)                                     # (16, 1024, 1026)
    xt_n = x.reshape(B, MQ, 4, 128).transpose(3, 0, 2, 1)  # (128, B, 4, MQ)
    xt = np.zeros((128, B, 4, MQP), np.float32)
    xt[:, :, :, :MQ] = xt_n
    xt = np.ascontiguousarray(xt)
    Ge = G[:, :, _KEEP]                                    # (16, 1024, 1024)
    g_all = Ge.reshape(D, KC, 128, FE).transpose(2, 1, 0, 3)  # (128,KC,D,FE)
    in_maps = [
        {
            "xt": xt,
            "g": np.ascontiguousarray(
                g_all[:, :, c * D_PER : (c + 1) * D_PER].reshape(
                    128, KC, D_PER * FE
                )
            ),
        }
        for c in range(NCORES)
    ]
    return in_maps


def _assemble(results):
    """per-core out2 (B, 4, 128, 4, NWP) -> (B, NW, D, NF) complex64."""
    full = np.zeros((B, NW, D, FW), dtype=np.float32)
    for c, r in enumerate(results):
        o = np.asarray(r["out"], dtype=np.float32)[..., :NW]
        o = o.transpose(0, 4, 1, 3, 2).reshape(B, NW, D_PER, FE)
        for dd in range(D_PER):
            full[:, :, c * D_PER + dd, _KEEP] = o[:, :, dd]
    return full.view(np.complex64).reshape(B, NW, D, NF)


# --------------------------------------------------------------------------
# runner (jitted multi-core executable, cached across kernel() calls)
# --------------------------------------------------------------------------
def _make_sharded(nc):
    import jax
    from jax.experimental.shard_map import shard_map
    from jax.sharding import Mesh, PartitionSpec

    from concourse import bass2jax as b2j
    import concourse.mybir as mybir

    b2j.install_neuronx_cc_hook()
    partition_name = nc.partition_id_tensor.name if nc.partition_id_tensor else None

    in_names, out_names, out_avals, zero_outs = [], [], [], []
    for alloc in nc.m.functions[0].allocations:
        if not isinstance(alloc, mybir.MemoryLocationSet):
            continue
        name = alloc.memorylocations[0].name
        if alloc.kind == "ExternalInput":
            if name != partition_name:
                in_names.append(name)
        elif alloc.kind == "ExternalOutput":
            out_names.append(name)
            shape = tuple(alloc.tensor_shape)
            dtype = mybir.dt.np(alloc.dtype)
            out_avals.append(jax.core.ShapedArray(shape, dtype))
            zero_outs.append(np.zeros(shape, dtype))
    all_names = in_names + out_names
    if partition_name is not None:
        all_names = all_names + [partition_name]

    def _body(*args):
        operands = list(args)
        if partition_name is not None:
            operands.append(b2j.partition_id_tensor())
        outs = b2j._bass_exec_p.bind(
            *operands,
            out_avals=tuple(out_avals),
            in_names=tuple(all_names),
            out_names=tuple(out_names),
            lowering_input_output_aliases=(),
            sim_require_finite=True,
            sim_require_nnan=True,
            nc=nc,
        )
        return tuple(outs)

    devices = jax.devices()[:NCORES]
    mesh = Mesh(np.asarray(devices), ("core",))
    nin = len(in_names) + len(zero_outs)
    sharded = jax.jit(
        shard_map(
            _body,
            mesh=mesh,
            in_specs=(PartitionSpec("core"),) * nin,
            out_specs=(PartitionSpec("core"),) * len(out_names),
            check_rep=False,
        ),
        keep_unused=True,
    )
    return sharded, in_names, out_names, out_avals, zero_outs


def _get_runner(iters):
    key = ("runner", iters)
    if key in _cache:
        return _cache[key]

    import jax

    nc = _get_nc(iters)
    sharded, in_names, out_names, out_avals, zero_outs = _make_sharded(nc)

    def call(in_maps):
        concat_in = [
            np.concatenate([in_maps[c][name] for c in range(NCORES)], axis=0)
            for name in in_names
        ] + [
            np.zeros((NCORES * z.shape[0], *z.shape[1:]), z.dtype)
            for z in zero_outs
        ]
        out_arrs = sharded(*concat_in)
        jax.block_until_ready(out_arrs)
        return [
            {
                name: np.asarray(out_arrs[i]).reshape(
                    NCORES, *out_avals[i].shape
                )[c]
                for i, name in enumerate(out_names)
            }
            for c in range(NCORES)
        ]

    _cache[key] = call
    return call


def kernel(x, dlnf, n_hann_splits=1, **_unused):
    in_maps = _prep_arrays(x, dlnf)
    iters = int(os.environ.get("KERNEL_ITERS", "1"))
    try:
        call = _get_runner(iters)
        results = call(in_maps)
    except Exception:
        # robust fallback: the reference implementation of the SPMD runner
        from concourse.bass_utils import run_bass_kernel_spmd

        nc = _get_nc(iters)
        res = run_bass_kernel_spmd(nc, in_maps, core_ids=list(range(NCORES)))
        results = res.results
    return _assemble(results)


# --------------------------------------------------------------------------
# benchmarking: jit once, time repeated executions (no retrace/relower)
# --------------------------------------------------------------------------
def prepare_bench(x, dlnf, iters):
    """Returns run() -> wall seconds for one execution of the iters-body NEFF."""
    import time

    import jax

    in_maps = _prep_arrays(x, dlnf)
    nc = _get_nc(iters)
    sharded, in_names, out_names, out_avals, zero_outs = _make_sharded(nc)
    concat_in = [
        np.concatenate([in_maps[c][name] for c in range(NCORES)], axis=0)
        for name in in_names
    ] + [np.zeros((NCORES * z.shape[0], *z.shape[1:]), z.dtype) for z in zero_outs]
    concat_in = [jax.device_put(a) for a in concat_in]

    out = sharded(*concat_in)
    jax.block_until_ready(out)

    def run():
        t0 = time.perf_counter()
        o = sharded(*concat_in)
        jax.block_until_ready(o)
        return time.perf_counter() - t0

    return run


if __name__ == "__main__":
    rng = np.random.default_rng(0)
    x = rng.standard_normal((B, N), dtype=np.float32)
    dlnf = rng.uniform(-0.5, 0.5, size=(D,)).astype(np.float32)
    out = kernel(x, dlnf, 1)
    print("out:", out.shape, out.dtype)



# revision 3
# speedup vs baseline: 1.4154x; 1.4154x over previous
"""DechirpSTFT Trainium2 kernel.

Math: the reference pipeline (hann window -> per-chirp lerp resample * jac
-> rfft(1024)) is linear in the windowed signal, so it folds into one
per-chirp matrix G_d[k, f2] (k = sample within window, f2 = interleaved
re/im of the 513 rfft bins).  The device kernel is the dense matmul

    out[f2, w] = sum_k G_d[k, f2] * x[b, 512*w + k]

computed G-stationary: each 128x128 tile of G is the PE weight operand and
all 512 (padded) windows stream through as the moving operand, contracting
k over 8 chunks of 128 into one PSUM bank per (b, f2-tile).

Dtype: bf16 operands (measured rel_rms 2.3e-3 vs the 2e-2 gate; fp8 e4m3
measured 2.7-3.7e-2 -> fails, so no DoubleRow double-pump).  bf16 streams
at the same 1 col/cycle as fp32r but halves DMA/SBUF traffic and allows
non-fp32 weight handling.  Output staged to bf16 (halves store traffic).

Schedule 'bf16ws' (weight-stationary): for each of the 16 weight tiles,
for each contraction chunk kc, the four batches stream b-inner so four
consecutive matmuls share one weight AP (walrus can elide reloads); each
(b, ft) PSUM bank accumulates one 8-chunk chain of N=512 columns.
Schedule 'base' is the previous fp32r h-outer variant kept for A/B.

Sharding: D=16 chirp rates, 2 per core across 8 cores (x replicated,
G sliced per core).  No cross-core communication.

Timing: no NTFF profiling is reachable through this axon client, so
test.py measures HW time as (wall(For_i(T)) - wall(For_i(1))) / (T-1)
with the jitted executable and device-resident inputs held across calls.
"""

import os
import sys

sys.path.insert(0, "/opt/trn_rl_repo")

import numpy as np
import ml_dtypes

# ---- problem constants (hardcoded; kernel.py must be self-contained) ----
B = 4
N = 262144
K = 1024
HOP = 512
NW = (N - K) // HOP + 1          # 511
KTAU = 1024
NF = 513                         # rfft bins
FW = 2 * NF                      # 1026 interleaved re/im
FE = 1024                        # live f2 cols per chirp (im0/imNyq dropped)
NWP = 512                        # padded window count
MQP = 513                        # padded m-quads so window 511 stays in-bounds
D = 16
NCORES = 8
D_PER = D // NCORES              # 2
KC = 8                           # contraction chunks of 128
MQ = N // 512                    # 512
NFT = 16                         # f2e weight tiles of 128 per core
EPS = 1e-8

SCHED = os.environ.get("KSCHED", "bf16ws")

_cache = {}


# --------------------------------------------------------------------------
# host-side G construction
# --------------------------------------------------------------------------
def _build_tables_np(dlnf):
    """Reference's per-chirp tables in numpy float32 (fallback path)."""
    dlnf = dlnf.astype(np.float32)
    beta = (2.0 * dlnf).astype(np.float32)
    small = np.abs(beta) < EPS
    beta_safe = np.where(small, np.float32(EPS), beta).astype(np.float32)
    e2b = np.exp(2.0 * beta_safe).astype(np.float32)

    tau = (2.0 * np.arange(KTAU, dtype=np.float32) / KTAU - 1.0).astype(np.float32)
    t_source = np.log(
        1.0 + (tau[None, :] + 1.0) / 2.0 * (e2b[:, None] - 1.0)
    ).astype(np.float32)
    t_source = (t_source / beta_safe[:, None] - 1.0).astype(np.float32)
    t_source = np.where(small[:, None], tau[None, :], t_source)

    tau_mid = np.float32(2.0 * (KTAU // 2) / KTAU - 1.0)
    t_mid = (
        np.log(1.0 + (tau_mid + 1.0) / 2.0 * (e2b - 1.0)) / beta_safe - 1.0
    ).astype(np.float32)
    t_mid = np.where(small, tau_mid, t_mid)

    jac = np.exp(-beta_safe[:, None] * (t_source - t_mid[:, None])).astype(np.float32)
    jac = np.where(small[:, None], np.float32(1.0), jac)

    idx = (np.float32(K / 2.0) * (t_source + 1.0)).astype(np.float32)
    idx_lo = np.clip(idx.astype(np.int32), 0, K - 2)
    frac = (idx - idx_lo.astype(np.float32)).astype(np.float32)
    return idx_lo, frac, jac


def _build_tables(dlnf):
    """Per-chirp tables, computed with jax on the CPU backend so the f32
    transcendentals (log/exp) match the reference bit-for-bit."""
    try:
        import jax
        import jax.numpy as jnp

        cpu = jax.devices("cpu")[0]
    except Exception:
        return _build_tables_np(dlnf)

    with jax.default_device(cpu):
        beta = 2.0 * jnp.asarray(dlnf, dtype=jnp.float32)
        small = jnp.abs(beta) < EPS
        beta_safe = jnp.where(small, EPS, beta)
        e2b = jnp.exp(2.0 * beta_safe)

        tau = 2.0 * jnp.arange(KTAU, dtype=jnp.float32) / KTAU - 1.0
        t_source = (
            jnp.log(1.0 + (tau[None, :] + 1.0) / 2.0 * (e2b[:, None] - 1.0))
            / beta_safe[:, None]
            - 1.0
        )
        t_source = jnp.where(small[:, None], tau[None, :], t_source)

        tau_mid = 2.0 * (KTAU // 2) / KTAU - 1.0
        t_mid = (
            jnp.log(1.0 + (tau_mid + 1.0) / 2.0 * (e2b - 1.0)) / beta_safe - 1.0
        )
        t_mid = jnp.where(small, tau_mid, t_mid)

        jac = jnp.exp(-beta_safe[:, None] * (t_source - t_mid[:, None]))
        jac = jnp.where(small[:, None], 1.0, jac)

        idx = (K / 2.0) * (t_source + 1.0)
        idx_lo = jnp.clip(idx.astype(jnp.int32), 0, K - 2)
        frac = idx - idx_lo.astype(jnp.float32)
    return np.asarray(idx_lo), np.asarray(frac), np.asarray(jac)


def _build_G(dlnf):
    """G[d, k, f2] f32: fused hann * lerp-resample * jac * rfft operator."""
    nd = dlnf.shape[0]
    idx_lo, frac, jac = _build_tables(dlnf)
    t = np.arange(KTAU, dtype=np.float64)
    f = np.arange(NF, dtype=np.float64)
    ang = 2.0 * np.pi * np.outer(t, f) / KTAU
    Wre = np.cos(ang)
    Wim = -np.sin(ang)
    n = np.arange(K, dtype=np.float32)
    hann = (0.5 * (1.0 - np.cos(2.0 * np.pi * n / K))).astype(np.float32)

    G = np.zeros((nd, K, FW), dtype=np.float64)
    for d in range(nd):
        c_lo = (jac[d] * (1.0 - frac[d])).astype(np.float64)
        c_hi = (jac[d] * frac[d]).astype(np.float64)
        Gre = np.zeros((K, NF))
        Gim = np.zeros((K, NF))
        np.add.at(Gre, idx_lo[d], c_lo[:, None] * Wre)
        np.add.at(Gim, idx_lo[d], c_lo[:, None] * Wim)
        np.add.at(Gre, idx_lo[d] + 1, c_hi[:, None] * Wre)
        np.add.at(Gim, idx_lo[d] + 1, c_hi[:, None] * Wim)
        G[d, :, 0::2] = Gre
        G[d, :, 1::2] = Gim
    G *= hann[None, :, None].astype(np.float64)
    return G.astype(np.float32)


# live G columns: drop im(bin0) (col 1) and im(Nyquist) (col 1025)
_KEEP = np.concatenate(([0], np.arange(2, 1025)))


# --------------------------------------------------------------------------
# device program
# --------------------------------------------------------------------------
def _build_nc(iters=1, sched=None):
    import concourse.bacc as bacc
    import concourse.mybir as mybir
    from concourse import tile

    sched = sched or SCHED
    f32 = mybir.dt.float32
    bf16 = mybir.dt.bfloat16
    mm_dt = mybir.dt.float32r if sched == "base" else bf16

    nc = bacc.Bacc("TRN2", target_bir_lowering=False, debug=False)

    # xt[p, b, r, mq] = x[b, 128*(4*mq + r) + p]  (mq innermost: every
    # matmul moving slice is contiguous)
    xt_d = nc.dram_tensor("xt", [128, B, 4, MQP], mm_dt, kind="ExternalInput")
    # g[p, kc, 1024*d + fe] = G_d[128*kc + p, keep[fe]]
    g_d = nc.dram_tensor("g", [128, KC, D_PER * FE], mm_dt, kind="ExternalInput")
    if sched == "base":
        out_d = nc.dram_tensor(
            "out", [B, NFT // 4, 128, 4, NWP], f32, kind="ExternalOutput"
        )
    else:
        # out[b, fg, p, j, w]: f2e tile ft = 2*fg + j, psum partition p
        out_d = nc.dram_tensor(
            "out", [B, NFT // 2, 128, 2, NWP], bf16, kind="ExternalOutput"
        )

    def body_base(nc, tc, xpool, gpool, spool, ppool):
        x_sb = xpool.tile([128, B, 4, MQP], mm_dt, name="x_sb")
        g_sb = gpool.tile([128, KC, D_PER * FE], mm_dt, name="g_sb")
        for b in range(B):
            nc.sync.dma_start(x_sb[:, b], xt_d[:, b])
        for kc in range(KC):
            nc.sync.dma_start(g_sb[:, kc], g_d[:, kc])

        for b in range(B):
            for ftp in range(NFT // 2):
                st = spool.tile([128, 2, NWP], f32, name="st")
                for jj in range(2):
                    ft = 2 * ftp + jj
                    ps = ppool.tile([128, NWP], f32, name="ps", tag="ps")
                    for h in range(2):
                        for kc in range(KC):
                            q, r = divmod(kc, 4)
                            nc.tensor.matmul(
                                ps[:, h * 256 : (h + 1) * 256],
                                g_sb[:, kc, 128 * ft : 128 * (ft + 1)],
                                x_sb[:, b, r, q + h * 256 : q + h * 256 + 256],
                                start=(kc == 0 and h == 0),
                                stop=(kc == KC - 1 and h == 1),
                            )
                    eng = nc.vector.tensor_copy if ft % 2 == 0 else nc.scalar.copy
                    eng(st[:, jj], ps)
                nc.scalar.dma_start(
                    out_d[b, ftp // 2, :, 2 * (ftp % 2) : 2 * (ftp % 2) + 2], st
                )

    def body_ws(nc, tc, xpool, gpool, spool, ppool):
        x_sb = xpool.tile([128, B, 4, MQP], mm_dt, name="x_sb")
        g_sb = gpool.tile([128, KC, D_PER * FE], mm_dt, name="g_sb")
        nc.sync.dma_start(g_sb[:, 0], g_d[:, 0])
        for b in range(B):
            nc.sync.dma_start(x_sb[:, b], xt_d[:, b])
        for kc in range(1, KC):
            nc.sync.dma_start(g_sb[:, kc], g_d[:, kc])

        st = {}
        for ftp in range(NFT):
            ps = [
                ppool.tile([128, NWP], f32, name=f"ps{b}", tag=f"ps{b}")
                for b in range(B)
            ]
            for kc in range(KC):
                q, r = divmod(kc, 4)
                w_ap = g_sb[:, kc, 128 * ftp : 128 * (ftp + 1)]
                for b in range(B):
                    nc.tensor.matmul(
                        ps[b],
                        w_ap,
                        x_sb[:, b, r, q : q + NWP],
                        start=(kc == 0),
                        stop=(kc == KC - 1),
                    )
            jj = ftp % 2
            if jj == 0:
                for b in range(B):
                    st[b] = spool.tile([128, 2, NWP], bf16, name=f"st{b}")
            for b in range(B):
                eng = nc.vector.tensor_copy if b % 2 == 0 else nc.scalar.copy
                eng(st[b][:, jj], ps[b])
            if jj == 1:
                for b in range(B):
                    nc.scalar.dma_start(out_d[b, ftp // 2], st[b])

    body = body_base if sched == "base" else body_ws

    with tile.TileContext(nc) as tc:
        with (
            tc.tile_pool(name="xsb", bufs=2) as xpool,
            tc.tile_pool(name="gsb", bufs=2) as gpool,
            tc.tile_pool(name="stage", bufs=3) as spool,
            tc.tile_pool(
                name="psum", bufs=8 if sched == "base" else 2, space="PSUM"
            ) as ppool,
        ):
            if iters > 1:
                with tc.For_i(0, iters, 1):
                    body(nc, tc, xpool, gpool, spool, ppool)
            else:
                body(nc, tc, xpool, gpool, spool, ppool)

    nc.compile()
    return nc


def _get_nc(iters=1, sched=None):
    sched = sched or SCHED
    key = ("nc", iters, sched)
    if key not in _cache:
        _cache[key] = _build_nc(iters, sched)
    return _cache[key]


# --------------------------------------------------------------------------
# host prep / assembly
# --------------------------------------------------------------------------
def _prep_arrays(x, dlnf, sched=None):
    """Host prep: G matrices + transposed/sharded device input arrays."""
    sched = sched or SCHED
    dt = np.float32 if sched == "base" else ml_dtypes.bfloat16
    x = np.asarray(x, dtype=np.float32)
    dlnf = np.asarray(dlnf, dtype=np.float32)
    G = _build_G(dlnf)                                     # (16, 1024, 1026)
    xt_n = x.reshape(B, MQ, 4, 128).transpose(3, 0, 2, 1)  # (128, B, 4, MQ)
    xt = np.zeros((128, B, 4, MQP), dt)
    xt[:, :, :, :MQ] = xt_n.astype(dt)
    xt = np.ascontiguousarray(xt)
    Ge = G[:, :, _KEEP]                                    # (16, 1024, 1024)
    g_all = Ge.reshape(D, KC, 128, FE).transpose(2, 1, 0, 3)  # (128,KC,D,FE)
    in_maps = [
        {
            "xt": xt,
            "g": np.ascontiguousarray(
                g_all[:, :, c * D_PER : (c + 1) * D_PER]
                .reshape(128, KC, D_PER * FE)
                .astype(dt)
            ),
        }
        for c in range(NCORES)
    ]
    return in_maps


def _assemble(results, sched=None):
    """per-core out2 -> (B, NW, D, NF) complex64."""
    sched = sched or SCHED
    full = np.zeros((B, NW, D, FW), dtype=np.float32)
    for c, r in enumerate(results):
        o = np.asarray(r["out"]).astype(np.float32)[..., :NW]
        o = o.transpose(0, 4, 1, 3, 2).reshape(B, NW, D_PER, FE)
        for dd in range(D_PER):
            full[:, :, c * D_PER + dd, _KEEP] = o[:, :, dd]
    return full.view(np.complex64).reshape(B, NW, D, NF)


# --------------------------------------------------------------------------
# runner (jitted multi-core executable, cached across kernel() calls)
# --------------------------------------------------------------------------
def _make_sharded(nc):
    import jax
    from jax.experimental.shard_map import shard_map
    from jax.sharding import Mesh, PartitionSpec

    from concourse import bass2jax as b2j
    import concourse.mybir as mybir

    b2j.install_neuronx_cc_hook()
    partition_name = nc.partition_id_tensor.name if nc.partition_id_tensor else None

    in_names, out_names, out_avals, zero_outs = [], [], [], []
    for alloc in nc.m.functions[0].allocations:
        if not isinstance(alloc, mybir.MemoryLocationSet):
            continue
        name = alloc.memorylocations[0].name
        if alloc.kind == "ExternalInput":
            if name != partition_name:
                in_names.append(name)
        elif alloc.kind == "ExternalOutput":
            out_names.append(name)
            shape = tuple(alloc.tensor_shape)
            dtype = mybir.dt.np(alloc.dtype)
            out_avals.append(jax.core.ShapedArray(shape, dtype))
            zero_outs.append(np.zeros(shape, dtype))
    all_names = in_names + out_names
    if partition_name is not None:
        all_names = all_names + [partition_name]

    def _body(*args):
        operands = list(args)
        if partition_name is not None:
            operands.append(b2j.partition_id_tensor())
        outs = b2j._bass_exec_p.bind(
            *operands,
            out_avals=tuple(out_avals),
            in_names=tuple(all_names),
            out_names=tuple(out_names),
            lowering_input_output_aliases=(),
            sim_require_finite=True,
            sim_require_nnan=True,
            nc=nc,
        )
        return tuple(outs)

    devices = jax.devices()[:NCORES]
    mesh = Mesh(np.asarray(devices), ("core",))
    nin = len(in_names) + len(zero_outs)
    sharded = jax.jit(
        shard_map(
            _body,
            mesh=mesh,
            in_specs=(PartitionSpec("core"),) * nin,
            out_specs=(PartitionSpec("core"),) * len(out_names),
            check_rep=False,
        ),
        keep_unused=True,
    )
    return sharded, in_names, out_names, out_avals, zero_outs


def _get_runner(iters, sched=None):
    sched = sched or SCHED
    key = ("runner", iters, sched)
    if key in _cache:
        return _cache[key]

    import jax

    nc = _get_nc(iters, sched)
    sharded, in_names, out_names, out_avals, zero_outs = _make_sharded(nc)

    def call(in_maps):
        concat_in = [
            np.concatenate([in_maps[c][name] for c in range(NCORES)], axis=0)
            for name in in_names
        ] + [
            np.zeros((NCORES * z.shape[0], *z.shape[1:]), z.dtype)
            for z in zero_outs
        ]
        out_arrs = sharded(*concat_in)
        jax.block_until_ready(out_arrs)
        return [
            {
                name: np.asarray(out_arrs[i]).reshape(
                    NCORES, *out_avals[i].shape
                )[c]
                for i, name in enumerate(out_names)
            }
            for c in range(NCORES)
        ]

    _cache[key] = call
    return call


def kernel(x, dlnf, n_hann_splits=1, **_unused):
    in_maps = _prep_arrays(x, dlnf)
    iters = int(os.environ.get("KERNEL_ITERS", "1"))
    try:
        call = _get_runner(iters)
        results = call(in_maps)
    except Exception:
        # robust fallback: the reference implementation of the SPMD runner
        from concourse.bass_utils import run_bass_kernel_spmd

        nc = _get_nc(iters)
        res = run_bass_kernel_spmd(nc, in_maps, core_ids=list(range(NCORES)))
        results = res.results
    return _assemble(results)


# --------------------------------------------------------------------------
# benchmarking: jit once, time repeated executions (no retrace/relower)
# --------------------------------------------------------------------------
def prepare_bench(x, dlnf, iters, sched=None):
    """Returns run() -> wall seconds for one execution of the iters-body NEFF."""
    import time

    import jax

    sched = sched or SCHED
    in_maps = _prep_arrays(x, dlnf, sched)
    nc = _get_nc(iters, sched)
    sharded, in_names, out_names, out_avals, zero_outs = _make_sharded(nc)
    concat_in = [
        np.concatenate([in_maps[c][name] for c in range(NCORES)], axis=0)
        for name in in_names
    ] + [np.zeros((NCORES * z.shape[0], *z.shape[1:]), z.dtype) for z in zero_outs]
    concat_in = [jax.device_put(a) for a in concat_in]

    out = sharded(*concat_in)
    jax.block_until_ready(out)

    def run():
        t0 = time.perf_counter()
        o = sharded(*concat_in)
        jax.block_until_ready(o)
        return time.perf_counter() - t0

    return run


if __name__ == "__main__":
    rng = np.random.default_rng(0)
    x = rng.standard_normal((B, N), dtype=np.float32)
    dlnf = rng.uniform(-0.5, 0.5, size=(D,)).astype(np.float32)
    out = kernel(x, dlnf, 1)
    print("out:", out.shape, out.dtype)
